# revision 11
# baseline (speedup 1.0000x reference)
"""Distributed Trainium2 Bass kernel for multi-head attention w/ RoPE.

Reference op (B=4, S=2048, D=1024, H=16, HD=64, fp32):
    q/k/v = hidden @ W{q,k,v}.T + b   (per-head reshape)
    q, k  = rope(q), rope(k)
    out   = softmax(q k^T / sqrt(HD)) v  @ Wo.T

Sharding: 8 cores = 4 batches x 2 query-halves. Each core computes the
K/V projections for its whole batch (duplicated across the half-pair --
this avoids every collective), Q projection + attention for its own 1024
queries, and the o-projection for its own output rows. Host-side unshard
is a pure concat. Per-core x^T is column-permuted so the core's own
queries always sit at columns 0:1024 (keeps the SPMD graph uniform);
K/V/rope tables follow the same permutation, which softmax+V is
invariant to.

Device layout is fully transposed (features on partitions): projections
produce Q^T/K^T, scores are computed as S^T per head with even/odd heads
of a pair issued back-to-back on disjoint PE row-groups (64-row tiling
mode, concurrent), exp runs on ACT over double-wide [128,1024] PSUM
tiles with the 1/sqrt(HD) scale folded in, attn@V uses natural V (from
an x^T-stationary projection) augmented with a ones column so the
softmax denominator falls out of the same matmul, and normalization
folds into the attn-out eviction via gpsimd partition-broadcast of a
fast-approx reciprocal row. RoPE = 2 DVE muls into bf16 + a 4-DMA
partition band-swap + one 2x-mode bf16 add. Nonzero biases are handled
through an augmented K=1 contraction row (ones x bias); the bias
matmuls are skipped when the caller's biases are all zero.
"""

import sys

import numpy as np

try:  # concourse ships in the container; fall back to the staged repo
    import concourse.bass  # noqa: F401
except Exception:  # pragma: no cover
    sys.path.insert(0, "/opt/trn_rl_repo")

import ml_dtypes

B, S, D, H = 4, 2048, 1024, 16
HD = D // H                      # 64
P = 128
NCORES = 8
SQ = S // 2                      # 1024 queries per core
SK = S                           # 2048 keys per core
ND = D // P                      # 8 feature chunks
NT = SK // P                     # 16 key/token chunks
QF = 512                         # matmul moving width
NQF = SQ // QF                   # 2
ROPE_BASE = 10000.0
BF16 = ml_dtypes.bfloat16

TRACE = False                    # test harness flips this
TRACE_KW = {}
LAST = {}                        # exec_time_ns / trace path for test harness

_cache = {}


def _build_nc(with_bias):
    import concourse.bass as bass
    import concourse.mybir as mybir
    import concourse.tile as tile
    from concourse import bacc
    from contextlib import ExitStack

    f32 = mybir.dt.float32
    bf16 = mybir.dt.bfloat16
    AF = mybir.ActivationFunctionType
    PSUM = bass.MemorySpace.PSUM

    nc = bacc.Bacc(None)
    xT = nc.declare_dram_parameter("xT", [D + 1, SK], bf16, False)
    wqT = nc.declare_dram_parameter("wqT", [D + 1, D], bf16, False)
    wkT = nc.declare_dram_parameter("wkT", [D + 1, D], bf16, False)
    wvT = nc.declare_dram_parameter("wvT", [D + 1, D], bf16, False)
    woT = nc.declare_dram_parameter("woT", [D, D], bf16, False)
    cosk = nc.declare_dram_parameter("cosk", [P, SK], bf16, False)
    sink = nc.declare_dram_parameter("sink", [P, SK], bf16, False)
    out = nc.declare_dram_parameter("out", [SQ, D], f32, True)

    with tile.TileContext(nc) as tc, ExitStack() as st:
        persist = st.enter_context(tc.tile_pool(name="persist", bufs=1))
        qt = [persist.tile([P, SQ], bf16, tag=f"qt{i}", name=f"qt{i}")
              for i in range(ND)]
        kt = [persist.tile([P, SK], bf16, tag=f"kt{i}", name=f"kt{i}")
              for i in range(ND)]
        vst = [persist.tile([P, H, HD + 1], bf16, tag=f"v{t}", name=f"v{t}")
               for t in range(NT)]
        at = [persist.tile([P, SQ], bf16, tag=f"at{i}", name=f"at{i}")
              for i in range(ND)]

        # ---------------- phase 1: projections + RoPE -------------------
        with ExitStack() as p1:
            sb1 = p1.enter_context(tc.tile_pool(name="ph1", bufs=1))
            wpool = p1.enter_context(tc.tile_pool(name="wp", bufs=2))
            tpool = p1.enter_context(tc.tile_pool(name="tmp", bufs=3))
            ps1 = p1.enter_context(tc.tile_pool(name="ps1", bufs=5, space=PSUM))

            wq = wpool.tile([P, ND, D], bf16, tag="w", name="w")
            nc.sync.dma_start(
                out=wq[:], in_=wqT[0:D, :].rearrange("(n p) o -> p n o", p=P))
            xs = [sb1.tile([P, SK], bf16, tag=f"x{d}", name=f"x{d}")
                  for d in range(ND)]
            for d_ in range(ND):
                nc.sync.dma_start(out=xs[d_][:], in_=xT[d_ * P:(d_ + 1) * P, :])
            ck = sb1.tile([P, SK], bf16, tag="ck", name="ck")
            sk_ = sb1.tile([P, SK], bf16, tag="sk", name="sk")
            nc.sync.dma_start(out=ck[:], in_=cosk[:, :])
            nc.sync.dma_start(out=sk_[:], in_=sink[:, :])
            if with_bias:
                xone = sb1.tile([1, SK], bf16, tag="xone", name="xone")
                nc.sync.dma_start(out=xone[:], in_=xT[D:D + 1, :])

            def qk_proj(w, wdram, outtiles, ntok):
                """outtiles[p][o, t] = rope(W @ x^T + b) for o-chunk p.

                Q's rope table is the leading [*, 0:SQ] slice of K's (the
                core's own tokens come first in the x^T permutation)."""
                if with_bias:
                    wb = wpool.tile([1, D], bf16, tag="wb", name="wb")
                    nc.sync.dma_start(out=wb[:], in_=wdram[D:D + 1, :])
                for p_ in range(ND):
                    for c in range(ntok // QF):
                        ps = ps1.tile([P, QF], f32, tag="pp", name="pp")
                        for d_ in range(ND):
                            nc.tensor.matmul(
                                ps[:], w[:, d_, p_ * P:(p_ + 1) * P],
                                xs[d_][:, c * QF:(c + 1) * QF],
                                start=(d_ == 0), stop=(not with_bias and d_ == ND - 1))
                        if with_bias:
                            nc.tensor.matmul(
                                ps[:], wb[:, p_ * P:(p_ + 1) * P],
                                xone[:, c * QF:(c + 1) * QF],
                                start=False, stop=True)
                        cs = ck[:, c * QF:(c + 1) * QF]
                        sn = sk_[:, c * QF:(c + 1) * QF]
                        t1 = tpool.tile([P, QF], bf16, tag="t1", name="t1")
                        t2 = tpool.tile([P, QF], bf16, tag="t2", name="t2")
                        t2s = tpool.tile([P, QF], bf16, tag="t2s", name="t2s")
                        nc.vector.tensor_mul(t1[:], ps[:], cs)
                        nc.vector.tensor_mul(t2[:], ps[:], sn)
                        # band swap d<->d+32 within each head via SBUF DMA
                        for b0 in (0, 64):
                            nc.sync.dma_start(
                                out=t2s[b0:b0 + 32, :], in_=t2[b0 + 32:b0 + 64, :])
                            nc.sync.dma_start(
                                out=t2s[b0 + 32:b0 + 64, :], in_=t2[b0:b0 + 32, :])
                        nc.vector.tensor_add(
                            outtiles[p_][:, c * QF:(c + 1) * QF], t1[:], t2s[:])

            qk_proj(wq, wqT, qt, SQ)
            wk = wpool.tile([P, ND, D], bf16, tag="w", name="w")
            nc.sync.dma_start(
                out=wk[:], in_=wkT[0:D, :].rearrange("(n p) o -> p n o", p=P))
            qk_proj(wk, wkT, kt, SK)

            # V in natural layout [tokens, feat] (x^T chunks stationary)
            wv = wpool.tile([P, ND, D], bf16, tag="w", name="w")
            nc.sync.dma_start(
                out=wv[:], in_=wvT[0:D, :].rearrange("(n p) o -> p n o", p=P))
            if with_bias:
                wvb = wpool.tile([1, D], bf16, tag="wb", name="wb")
                nc.sync.dma_start(out=wvb[:], in_=wvT[D:D + 1, :])
            for t_ in range(NT):
                for oh in range(2):
                    ps = ps1.tile([P, QF], f32, tag="pp", name="pp")
                    for d_ in range(ND):
                        nc.tensor.matmul(
                            ps[:], xs[d_][:, t_ * P:(t_ + 1) * P],
                            wv[:, d_, oh * QF:(oh + 1) * QF],
                            start=(d_ == 0), stop=(not with_bias and d_ == ND - 1))
                    if with_bias:
                        nc.tensor.matmul(
                            ps[:], xone[:, t_ * P:(t_ + 1) * P],
                            wvb[:, oh * QF:(oh + 1) * QF],
                            start=False, stop=True)
                    nc.scalar.activation(
                        vst[t_][:, oh * 8:(oh + 1) * 8, 0:HD],
                        ps[:].rearrange("p (h d) -> p h d", d=HD), AF.Copy)
                nc.vector.memset(vst[t_][:, :, HD:HD + 1], 1.0)

        # ---------------- phase 2: attention ---------------------------
        with ExitStack() as p2:
            etp = p2.enter_context(tc.tile_pool(name="et", bufs=20))
            npool = p2.enter_context(tc.tile_pool(name="nrm", bufs=4))
            ps_s = p2.enter_context(tc.tile_pool(name="pss", bufs=3, space=PSUM))
            ps_o = p2.enter_context(tc.tile_pool(name="pso", bufs=2, space=PSUM))
            for pi in range(ND):
                for qh in range(NQF):
                    qs = slice(qh * QF, (qh + 1) * QF)
                    ete, eto = [], []
                    for kcp in range(NT // 2):
                        # even/odd heads on disjoint PE row groups -> the
                        # T0/T8 pairs execute concurrently in 64-row mode
                        spe = ps_s.tile([P, 2 * QF], f32, tag="s", name="s")
                        spo = ps_s.tile([P, 2 * QF], f32, tag="s", name="s")
                        for j in range(2):
                            ks_ = slice((2 * kcp + j) * P, (2 * kcp + j + 1) * P)
                            js = slice(j * QF, (j + 1) * QF)
                            nc.tensor.matmul(
                                spe[:, js], kt[pi][0:64, ks_],
                                qt[pi][0:64, qs], start=True, stop=True)
                            nc.tensor.matmul(
                                spo[:, js], kt[pi][64:128, ks_],
                                qt[pi][64:128, qs], start=True, stop=True)
                        ee = etp.tile([P, 2 * QF], bf16, tag="e", name="e")
                        eo = etp.tile([P, 2 * QF], bf16, tag="e", name="e")
                        nc.scalar.activation(ee[:], spe[:], AF.Exp, scale=0.125)
                        nc.scalar.activation(eo[:], spo[:], AF.Exp, scale=0.125)
                        ete.append(ee)
                        eto.append(eo)
                    for h, ets in ((2 * pi, ete), (2 * pi + 1, eto)):
                        b0 = 64 * (h % 2)
                        op = ps_o.tile([HD + 1, QF], f32, tag="o", name="o")
                        for kc in range(NT):
                            nc.tensor.matmul(
                                op[:], vst[kc][:, h, :],
                                ets[kc // 2][:, (kc % 2) * QF:(kc % 2 + 1) * QF],
                                start=(kc == 0), stop=(kc == NT - 1))
                        # approx reciprocal base-aligned at p64, DMA-hop to
                        # p0, gpsimd-broadcast, normalize on PSUM eviction
                        sm = npool.tile([HD + 1, QF], f32, tag="sm", name="sm")
                        nc.vector.reciprocal(sm[HD:HD + 1, :], op[HD:HD + 1, :])
                        rc = npool.tile([1, QF], f32, tag="rc", name="rc")
                        nc.sync.dma_start(out=rc[:], in_=sm[HD:HD + 1, :])
                        bc = npool.tile([HD, QF], f32, tag="bc", name="bc")
                        nc.gpsimd.partition_broadcast(bc[:], rc[:])
                        nc.vector.tensor_mul(
                            at[pi][b0:b0 + 64, qs], op[0:HD, :], bc[:])

        # ---------------- phase 3: o-projection ------------------------
        with ExitStack() as p3:
            wop = p3.enter_context(tc.tile_pool(name="wo", bufs=1))
            outp = p3.enter_context(tc.tile_pool(name="ou", bufs=4))
            ps3 = p3.enter_context(tc.tile_pool(name="ps3", bufs=4, space=PSUM))
            wo = wop.tile([P, ND, D], bf16, tag="wo", name="wo")
            nc.sync.dma_start(
                out=wo[:], in_=woT[:, :].rearrange("(n p) o -> p n o", p=P))
            for qc in range(ND):
                for oh in range(2):
                    ps = ps3.tile([P, QF], f32, tag="p3", name="p3")
                    for f in range(ND):
                        nc.tensor.matmul(
                            ps[:], at[f][:, qc * P:(qc + 1) * P],
                            wo[:, f, oh * QF:(oh + 1) * QF],
                            start=(f == 0), stop=(f == ND - 1))
                    ob = outp.tile([P, QF], f32, tag="ob", name="ob")
                    nc.scalar.activation(ob[:], ps[:], AF.Copy)
                    nc.sync.dma_start(
                        out=out[qc * P:(qc + 1) * P, oh * QF:(oh + 1) * QF],
                        in_=ob[:])
    nc.compile()
    return nc


def _rope_tables(pos):
    """pos [n] -> (cos [128, n] bf16, sign-folded sin [128, n] bf16)."""
    inv = ROPE_BASE ** (-np.arange(0, HD, 2, dtype=np.float64) / HD)
    fr = np.outer(pos.astype(np.float64), inv)          # [n, 32]
    c, s = np.cos(fr), np.sin(fr)
    cos64 = np.concatenate([c, c], axis=1).T            # [64, n]
    sinA = np.concatenate([s, -s], axis=1).T            # [64, n]
    return (np.tile(cos64, (2, 1)).astype(BF16),
            np.tile(sinA, (2, 1)).astype(BF16))


def _aug_w(w, b):
    """[D, D] weight + [D] bias -> bf16 [D+1, D] (W.T with bias row)."""
    wa = np.empty((D + 1, D), dtype=np.float32)
    wa[:D] = np.asarray(w, dtype=np.float32).T
    wa[D] = np.asarray(b, dtype=np.float32)
    return np.ascontiguousarray(wa).astype(BF16)


def kernel(hidden_states, position_ids, Wq, bq, Wk, bk, Wv, bv, Wo):
    from concourse import bass_utils

    with_bias = bool(
        np.any(np.asarray(bq)) or np.any(np.asarray(bk)) or np.any(np.asarray(bv)))
    key = ("nc", with_bias)
    if key not in _cache:
        _cache[key] = _build_nc(with_bias)
    nc = _cache[key]

    hs = np.asarray(hidden_states, dtype=np.float32)
    pos = np.asarray(position_ids)
    wq = _aug_w(Wq, bq)
    wk = _aug_w(Wk, bk)
    wv = _aug_w(Wv, bv)
    wo = np.ascontiguousarray(np.asarray(Wo, dtype=np.float32).T).astype(BF16)

    in_maps = []
    for core in range(NCORES):
        b, hf = core // 2, core % 2
        perm = np.concatenate([
            np.arange(hf * SQ, (hf + 1) * SQ),
            np.arange((1 - hf) * SQ, (2 - hf) * SQ)])
        xp = hs[b][perm]                                 # [S, D], own half first
        xT = np.empty((D + 1, SK), dtype=np.float32)
        xT[:D] = xp.T
        xT[D] = 1.0
        ck, sk = _rope_tables(np.asarray(pos[b][perm]))
        in_maps.append({
            "xT": xT.astype(BF16), "wqT": wq, "wkT": wk, "wvT": wv, "woT": wo,
            "cosk": ck, "sink": sk,
        })

    res = bass_utils.run_bass_kernel_spmd(
        nc, in_maps, core_ids=list(range(NCORES)), trace=TRACE, **TRACE_KW)
    LAST["exec_time_ns"] = res.exec_time_ns
    LAST["mean_exec_time_ns"] = res.mean_exec_time_ns
    LAST["trace"] = res.instructions_and_trace
    LAST["profile_json"] = res.profile_json

    outp = np.empty((B, S, D), dtype=np.float32)
    for core in range(NCORES):
        b, hf = core // 2, core % 2
        outp[b, hf * SQ:(hf + 1) * SQ] = res.results[core]["out"]
    return outp


# revision 12
# speedup vs baseline: 1.0393x; 1.0393x over previous
"""Distributed Trainium2 Bass kernel for multi-head attention w/ RoPE.

Reference op (B=4, S=2048, D=1024, H=16, HD=64, fp32):
    q/k/v = hidden @ W{q,k,v}.T + b   (per-head reshape)
    q, k  = rope(q), rope(k)
    out   = softmax(q k^T / sqrt(HD)) v  @ Wo.T

Sharding: 8 cores = 4 batches x 2 query-halves. Each core computes the
K/V projections for its whole batch (duplicated across the half-pair --
this avoids every collective), Q projection + attention for its own 1024
queries, and the o-projection for its own output rows. Host-side unshard
is a pure concat. Per-core x^T is column-permuted so the core's own
queries always sit at columns 0:1024 (keeps the SPMD graph uniform);
K/V/rope tables follow the same permutation, which softmax+V is
invariant to.

Device layout is fully transposed (features on partitions): projections
produce Q^T/K^T, scores are computed as S^T per head with even/odd heads
of a pair issued back-to-back on disjoint PE row-groups (64-row tiling
mode, concurrent), exp runs on ACT over double-wide [128,1024] PSUM
tiles with the 1/sqrt(HD) scale folded in, attn@V uses natural V (from
an x^T-stationary projection) augmented with a ones column so the
softmax denominator falls out of the same matmul, and normalization
folds into the attn-out eviction via gpsimd partition-broadcast of a
fast-approx reciprocal row. RoPE = 2 DVE muls into bf16 + a 4-DMA
partition band-swap + one 2x-mode bf16 add. Nonzero biases are handled
through an augmented K=1 contraction row (ones x bias); the bias
matmuls are skipped when the caller's biases are all zero.
"""

import sys

import numpy as np

try:  # concourse ships in the container; fall back to the staged repo
    import concourse.bass  # noqa: F401
except Exception:  # pragma: no cover
    sys.path.insert(0, "/opt/trn_rl_repo")

import ml_dtypes

B, S, D, H = 4, 2048, 1024, 16
HD = D // H                      # 64
P = 128
NCORES = 8
SQ = S // 2                      # 1024 queries per core
SK = S                           # 2048 keys per core
ND = D // P                      # 8 feature chunks
NT = SK // P                     # 16 key/token chunks
QF = 512                         # matmul moving width
NQF = SQ // QF                   # 2
ROPE_BASE = 10000.0
BF16 = ml_dtypes.bfloat16

TRACE = False                    # test harness flips this
TRACE_KW = {}
LAST = {}                        # exec_time_ns / trace path for test harness

_cache = {}


def _build_nc(with_bias):
    import concourse.bass as bass
    import concourse.mybir as mybir
    import concourse.tile as tile
    from concourse import bacc
    from contextlib import ExitStack

    f32 = mybir.dt.float32
    bf16 = mybir.dt.bfloat16
    AF = mybir.ActivationFunctionType
    PSUM = bass.MemorySpace.PSUM

    nc = bacc.Bacc(None)
    xT = nc.declare_dram_parameter("xT", [D + 1, SK], bf16, False)
    wqT = nc.declare_dram_parameter("wqT", [D + 1, D], bf16, False)
    wkT = nc.declare_dram_parameter("wkT", [D + 1, D], bf16, False)
    wvT = nc.declare_dram_parameter("wvT", [D + 1, D], bf16, False)
    woT = nc.declare_dram_parameter("woT", [D, D], bf16, False)
    cosk = nc.declare_dram_parameter("cosk", [P, SK], bf16, False)
    sink = nc.declare_dram_parameter("sink", [P, SK], bf16, False)
    out = nc.declare_dram_parameter("out", [SQ, D], f32, True)

    with tile.TileContext(nc) as tc, ExitStack() as st:
        persist = st.enter_context(tc.tile_pool(name="persist", bufs=1))
        qt = [persist.tile([P, SQ], bf16, tag=f"qt{i}", name=f"qt{i}")
              for i in range(ND)]
        kt = [persist.tile([P, SK], bf16, tag=f"kt{i}", name=f"kt{i}")
              for i in range(ND)]
        vst = [persist.tile([P, H, HD + 1], bf16, tag=f"v{t}", name=f"v{t}")
               for t in range(NT)]

        # ---------------- phase 1: projections + RoPE -------------------
        with ExitStack() as p1:
            sb1 = p1.enter_context(tc.tile_pool(name="ph1", bufs=1))
            wpool = p1.enter_context(tc.tile_pool(name="wp", bufs=2))
            tpool = p1.enter_context(tc.tile_pool(name="tmp", bufs=3))
            ps1 = p1.enter_context(tc.tile_pool(name="ps1", bufs=5, space=PSUM))

            wq = wpool.tile([P, ND, D], bf16, tag="w", name="w")
            nc.sync.dma_start(
                out=wq[:], in_=wqT[0:D, :].rearrange("(n p) o -> p n o", p=P))
            xs = [sb1.tile([P, SK], bf16, tag=f"x{d}", name=f"x{d}")
                  for d in range(ND)]
            for d_ in range(ND):
                nc.sync.dma_start(out=xs[d_][:], in_=xT[d_ * P:(d_ + 1) * P, :])
            ck = sb1.tile([P, SK], bf16, tag="ck", name="ck")
            sk_ = sb1.tile([P, SK], bf16, tag="sk", name="sk")
            nc.sync.dma_start(out=ck[:], in_=cosk[:, :])
            nc.sync.dma_start(out=sk_[:], in_=sink[:, :])
            if with_bias:
                xone = sb1.tile([1, SK], bf16, tag="xone", name="xone")
                nc.sync.dma_start(out=xone[:], in_=xT[D:D + 1, :])

            def qk_proj(w, wdram, outtiles, ntok):
                """outtiles[p][o, t] = rope(W @ x^T + b) for o-chunk p.

                Q's rope table is the leading [*, 0:SQ] slice of K's (the
                core's own tokens come first in the x^T permutation)."""
                if with_bias:
                    wb = wpool.tile([1, D], bf16, tag="wb", name="wb")
                    nc.sync.dma_start(out=wb[:], in_=wdram[D:D + 1, :])
                for p_ in range(ND):
                    t1 = tpool.tile([P, ntok], bf16, tag="t1", name="t1")
                    t2 = tpool.tile([P, ntok], bf16, tag="t2", name="t2")
                    t2s = tpool.tile([P, ntok], bf16, tag="t2s", name="t2s")
                    for c in range(ntok // QF):
                        ps = ps1.tile([P, QF], f32, tag="pp", name="pp")
                        for d_ in range(ND):
                            nc.tensor.matmul(
                                ps[:], w[:, d_, p_ * P:(p_ + 1) * P],
                                xs[d_][:, c * QF:(c + 1) * QF],
                                start=(d_ == 0), stop=(not with_bias and d_ == ND - 1))
                        if with_bias:
                            nc.tensor.matmul(
                                ps[:], wb[:, p_ * P:(p_ + 1) * P],
                                xone[:, c * QF:(c + 1) * QF],
                                start=False, stop=True)
                        cslice = slice(c * QF, (c + 1) * QF)
                        nc.vector.tensor_mul(t1[:, cslice], ps[:], ck[:, cslice])
                        nc.vector.tensor_mul(t2[:, cslice], ps[:], sk_[:, cslice])
                    # band swap d<->d+32 within each head, batched full-width
                    # (dispatched on the idle scalar engine's DMA queue)
                    for b0 in (0, 64):
                        nc.scalar.dma_start(
                            out=t2s[b0:b0 + 32, :], in_=t2[b0 + 32:b0 + 64, :])
                        nc.scalar.dma_start(
                            out=t2s[b0 + 32:b0 + 64, :], in_=t2[b0:b0 + 32, :])
                    nc.vector.tensor_add(outtiles[p_][:], t1[:], t2s[:])

            qk_proj(wq, wqT, qt, SQ)
            wk = wpool.tile([P, ND, D], bf16, tag="w", name="w")
            nc.sync.dma_start(
                out=wk[:], in_=wkT[0:D, :].rearrange("(n p) o -> p n o", p=P))
            qk_proj(wk, wkT, kt, SK)

            # V in natural layout [tokens, feat] (x^T chunks stationary)
            wv = wpool.tile([P, ND, D], bf16, tag="w", name="w")
            nc.sync.dma_start(
                out=wv[:], in_=wvT[0:D, :].rearrange("(n p) o -> p n o", p=P))
            if with_bias:
                wvb = wpool.tile([1, D], bf16, tag="wb", name="wb")
                nc.sync.dma_start(out=wvb[:], in_=wvT[D:D + 1, :])
            for t_ in range(NT):
                for oh in range(2):
                    ps = ps1.tile([P, QF], f32, tag="pp", name="pp")
                    for d_ in range(ND):
                        nc.tensor.matmul(
                            ps[:], xs[d_][:, t_ * P:(t_ + 1) * P],
                            wv[:, d_, oh * QF:(oh + 1) * QF],
                            start=(d_ == 0), stop=(not with_bias and d_ == ND - 1))
                    if with_bias:
                        nc.tensor.matmul(
                            ps[:], xone[:, t_ * P:(t_ + 1) * P],
                            wvb[:, oh * QF:(oh + 1) * QF],
                            start=False, stop=True)
                    nc.scalar.activation(
                        vst[t_][:, oh * 8:(oh + 1) * 8, 0:HD],
                        ps[:].rearrange("p (h d) -> p h d", d=HD), AF.Copy)
                nc.vector.memset(vst[t_][:, :, HD:HD + 1], 1.0)

        # ------- phase 2: attention + interleaved o-projection ---------
        with ExitStack() as p2:
            late = p2.enter_context(tc.tile_pool(name="late", bufs=1))
            etp = p2.enter_context(tc.tile_pool(name="et", bufs=20))
            npool = p2.enter_context(tc.tile_pool(name="nrm", bufs=4))
            outp = p2.enter_context(tc.tile_pool(name="ou", bufs=4))
            ps_s = p2.enter_context(tc.tile_pool(name="pss", bufs=2, space=PSUM))
            ps_o = p2.enter_context(tc.tile_pool(name="pso", bufs=2, space=PSUM))
            ps3 = p2.enter_context(tc.tile_pool(name="ps3", bufs=2, space=PSUM))
            at = [late.tile([P, SQ], bf16, tag=f"at{i}", name=f"at{i}")
                  for i in range(ND)]
            wo = late.tile([P, ND, D], bf16, tag="wo", name="wo")
            nc.sync.dma_start(
                out=wo[:], in_=woT[:, :].rearrange("(n p) o -> p n o", p=P))

            def oproj(qc):
                for oh in range(2):
                    ps = ps3.tile([P, QF], f32, tag="p3", name="p3")
                    for f in range(ND):
                        nc.tensor.matmul(
                            ps[:], at[f][:, qc * P:(qc + 1) * P],
                            wo[:, f, oh * QF:(oh + 1) * QF],
                            start=(f == 0), stop=(f == ND - 1))
                    ob = outp.tile([P, QF], f32, tag="ob", name="ob")
                    nc.scalar.activation(ob[:], ps[:], AF.Copy)
                    nc.sync.dma_start(
                        out=out[qc * P:(qc + 1) * P, oh * QF:(oh + 1) * QF],
                        in_=ob[:])

            for qh in range(NQF):
                qs = slice(qh * QF, (qh + 1) * QF)
                for pi in range(ND):
                    ete, eto = [], []
                    for kcp in range(NT // 2):
                        # even/odd heads on disjoint PE row groups -> the
                        # T0/T8 pairs execute concurrently in 64-row mode
                        spe = ps_s.tile([P, 2 * QF], f32, tag="s", name="s")
                        spo = ps_s.tile([P, 2 * QF], f32, tag="s", name="s")
                        for j in range(2):
                            ks_ = slice((2 * kcp + j) * P, (2 * kcp + j + 1) * P)
                            js = slice(j * QF, (j + 1) * QF)
                            nc.tensor.matmul(
                                spe[:, js], kt[pi][0:64, ks_],
                                qt[pi][0:64, qs], start=True, stop=True)
                            nc.tensor.matmul(
                                spo[:, js], kt[pi][64:128, ks_],
                                qt[pi][64:128, qs], start=True, stop=True)
                        ee = etp.tile([P, 2 * QF], bf16, tag="e", name="e")
                        eo = etp.tile([P, 2 * QF], bf16, tag="e", name="e")
                        nc.scalar.activation(ee[:], spe[:], AF.Exp, scale=0.125)
                        nc.scalar.activation(eo[:], spo[:], AF.Exp, scale=0.125)
                        ete.append(ee)
                        eto.append(eo)
                    for h, ets in ((2 * pi, ete), (2 * pi + 1, eto)):
                        op = ps_o.tile([HD + 1, QF], f32, tag="o", name="o")
                        for kc in range(NT):
                            nc.tensor.matmul(
                                op[:], vst[kc][:, h, :],
                                ets[kc // 2][:, (kc % 2) * QF:(kc % 2 + 1) * QF],
                                start=(kc == 0), stop=(kc == NT - 1))
                        # evict PSUM immediately; normalize fully in SBUF:
                        # exact reciprocal base-aligned at p64, gpsimd DMA-hop
                        # to p0, partition-broadcast, multiply
                        osb = npool.tile([HD + 1, QF], f32, tag="osb", name="osb")
                        nc.scalar.activation(osb[:], op[:], AF.Copy)
                        sm = npool.tile([HD + 1, QF], f32, tag="sm", name="sm")
                        nc.vector.reciprocal(sm[HD:HD + 1, :], osb[HD:HD + 1, :])
                        rc = npool.tile([1, QF], f32, tag="rc", name="rc")
                        nc.gpsimd.dma_start(out=rc[:], in_=sm[HD:HD + 1, :])
                        bc = npool.tile([HD, QF], f32, tag="bc", name="bc")
                        nc.gpsimd.partition_broadcast(bc[:], rc[:])
                        if h % 2 == 0:
                            nc.vector.tensor_mul(
                                at[pi][0:64, qs], osb[0:HD, :], bc[:])
                        else:
                            # odd heads land at partition base 64; a pure-SBUF
                            # base-shifted DVE write corrupts, so write at
                            # base 0 and DMA-hop into place
                            atmp = npool.tile([HD, QF], bf16, tag="atm", name="atm")
                            nc.vector.tensor_mul(atmp[:], osb[0:HD, :], bc[:])
                            nc.gpsimd.dma_start(
                                out=at[pi][64:128, qs], in_=atmp[:])
                # o-projection for the q-chunks this qh pass completed
                for qc in range(qh * ND // 2, (qh + 1) * ND // 2):
                    oproj(qc)
    nc.compile()
    return nc


def _rope_tables(pos):
    """pos [n] -> (cos [128, n] bf16, sign-folded sin [128, n] bf16)."""
    inv = ROPE_BASE ** (-np.arange(0, HD, 2, dtype=np.float64) / HD)
    fr = np.outer(pos.astype(np.float64), inv)          # [n, 32]
    c, s = np.cos(fr), np.sin(fr)
    cos64 = np.concatenate([c, c], axis=1).T            # [64, n]
    sinA = np.concatenate([s, -s], axis=1).T            # [64, n]
    return (np.tile(cos64, (2, 1)).astype(BF16),
            np.tile(sinA, (2, 1)).astype(BF16))


def _aug_w(w, b):
    """[D, D] weight + [D] bias -> bf16 [D+1, D] (W.T with bias row)."""
    wa = np.empty((D + 1, D), dtype=np.float32)
    wa[:D] = np.asarray(w, dtype=np.float32).T
    wa[D] = np.asarray(b, dtype=np.float32)
    return np.ascontiguousarray(wa).astype(BF16)


def kernel(hidden_states, position_ids, Wq, bq, Wk, bk, Wv, bv, Wo):
    from concourse import bass_utils

    with_bias = bool(
        np.any(np.asarray(bq)) or np.any(np.asarray(bk)) or np.any(np.asarray(bv)))
    key = ("nc", with_bias)
    if key not in _cache:
        _cache[key] = _build_nc(with_bias)
    nc = _cache[key]

    hs = np.asarray(hidden_states, dtype=np.float32)
    pos = np.asarray(position_ids)
    wq = _aug_w(Wq, bq)
    wk = _aug_w(Wk, bk)
    wv = _aug_w(Wv, bv)
    wo = np.ascontiguousarray(np.asarray(Wo, dtype=np.float32).T).astype(BF16)

    in_maps = []
    for core in range(NCORES):
        b, hf = core // 2, core % 2
        perm = np.concatenate([
            np.arange(hf * SQ, (hf + 1) * SQ),
            np.arange((1 - hf) * SQ, (2 - hf) * SQ)])
        xp = hs[b][perm]                                 # [S, D], own half first
        xT = np.empty((D + 1, SK), dtype=np.float32)
        xT[:D] = xp.T
        xT[D] = 1.0
        ck, sk = _rope_tables(np.asarray(pos[b][perm]))
        in_maps.append({
            "xT": xT.astype(BF16), "wqT": wq, "wkT": wk, "wvT": wv, "woT": wo,
            "cosk": ck, "sink": sk,
        })

    res = bass_utils.run_bass_kernel_spmd(
        nc, in_maps, core_ids=list(range(NCORES)), trace=TRACE, **TRACE_KW)
    LAST["exec_time_ns"] = res.exec_time_ns
    LAST["mean_exec_time_ns"] = res.mean_exec_time_ns
    LAST["trace"] = res.instructions_and_trace
    LAST["profile_json"] = res.profile_json

    outp = np.empty((B, S, D), dtype=np.float32)
    for core in range(NCORES):
        b, hf = core // 2, core % 2
        outp[b, hf * SQ:(hf + 1) * SQ] = res.results[core]["out"]
    return outp


# revision 13
# speedup vs baseline: 1.0529x; 1.0131x over previous
"""Distributed Trainium2 Bass kernel for multi-head attention w/ RoPE.

Reference op (B=4, S=2048, D=1024, H=16, HD=64, fp32):
    q/k/v = hidden @ W{q,k,v}.T + b   (per-head reshape)
    q, k  = rope(q), rope(k)
    out   = softmax(q k^T / sqrt(HD)) v  @ Wo.T

Sharding: 8 cores = 4 batches x 2 query-halves. Each core computes the
K/V projections for its whole batch (duplicated across the half-pair --
this avoids every collective), Q projection + attention for its own 1024
queries, and the o-projection for its own output rows. Host-side unshard
is a pure concat. Per-core x^T is column-permuted so the core's own
queries always sit at columns 0:1024 (keeps the SPMD graph uniform);
K/V/rope tables follow the same permutation, which softmax+V is
invariant to.

Single fused pipeline, fully transposed layout (features on partitions):
V projects first (natural layout, ones column appended so the softmax
denominator falls out of the attn@V matmul); then per head-pair: Q^T/K^T
projection chunks -> RoPE (DVE muls + a batched DMA partition band-swap
+ one 2x bf16 add) -> scores S^T with even/odd heads issued on disjoint
PE row-groups (64-row tiling, concurrent) -> wide [128,1024] exp on ACT
with the 1/sqrt(HD) scale folded in -> attn@V accumulation interleaved
at k-chunk granularity so PE fills ACT's exp latency. Normalization is
evicted-early (DVE copy frees PSUM), exact reciprocal runs base-aligned,
gpsimd hops/broadcasts it, and odd heads write via a DMA partition hop.
The o-projection consumes the transposed attention output directly.
Nonzero biases ride an augmented K=1 contraction row (skipped when the
caller's biases are all zero). All matmuls bf16, fp32 accumulation.
"""

import sys

import numpy as np

try:  # concourse ships in the container; fall back to the staged repo
    import concourse.bass  # noqa: F401
except Exception:  # pragma: no cover
    sys.path.insert(0, "/opt/trn_rl_repo")

import ml_dtypes

B, S, D, H = 4, 2048, 1024, 16
HD = D // H                      # 64
P = 128
NCORES = 8
SQ = S // 2                      # 1024 queries per core
SK = S                           # 2048 keys per core
ND = D // P                      # 8 feature chunks
NT = SK // P                     # 16 key/token chunks
QF = 512                         # matmul moving width
NQF = SQ // QF                   # 2
ROPE_BASE = 10000.0
BF16 = ml_dtypes.bfloat16

TRACE = False                    # test harness flips this
TRACE_KW = {}
LAST = {}                        # exec_time_ns / trace path for test harness

_cache = {}


def _build_nc(with_bias):
    import concourse.bass as bass
    import concourse.mybir as mybir
    import concourse.tile as tile
    from concourse import bacc
    from contextlib import ExitStack

    f32 = mybir.dt.float32
    bf16 = mybir.dt.bfloat16
    AF = mybir.ActivationFunctionType
    PSUM = bass.MemorySpace.PSUM

    nc = bacc.Bacc(None)
    xT = nc.declare_dram_parameter("xT", [D + 1, SK], bf16, False)
    wqT = nc.declare_dram_parameter("wqT", [D + 1, D], bf16, False)
    wkT = nc.declare_dram_parameter("wkT", [D + 1, D], bf16, False)
    wvT = nc.declare_dram_parameter("wvT", [D + 1, D], bf16, False)
    woT = nc.declare_dram_parameter("woT", [D, D], bf16, False)
    cosk = nc.declare_dram_parameter("cosk", [P, SK], bf16, False)
    sink = nc.declare_dram_parameter("sink", [P, SK], bf16, False)
    out = nc.declare_dram_parameter("out", [SQ, D], f32, True)

    with tile.TileContext(nc) as tc, ExitStack() as st:
        sb = st.enter_context(tc.tile_pool(name="sb", bufs=1))
        qk = st.enter_context(tc.tile_pool(name="qk", bufs=3))
        wp = st.enter_context(tc.tile_pool(name="wp", bufs=2))
        tp = st.enter_context(tc.tile_pool(name="tp", bufs=2))
        etp = st.enter_context(tc.tile_pool(name="et", bufs=6))
        npool = st.enter_context(tc.tile_pool(name="nrm", bufs=3))
        outp = st.enter_context(tc.tile_pool(name="ou", bufs=2))
        psp = st.enter_context(tc.tile_pool(name="ps", bufs=2, space=PSUM))

        vst = [sb.tile([P, H, HD + 1], bf16, tag=f"v{t}", name=f"v{t}")
               for t in range(NT)]
        at = [sb.tile([P, SQ], bf16, tag=f"at{i}", name=f"at{i}")
              for i in range(ND)]

        # ---- loads -----------------------------------------------------
        wv = wp.tile([P, ND, D], bf16, tag="wbig", name="wv", bufs=1)
        nc.sync.dma_start(
            out=wv[:], in_=wvT[0:D, :].rearrange("(n p) o -> p n o", p=P))
        xs = [sb.tile([P, SK], bf16, tag=f"x{d}", name=f"x{d}")
              for d in range(ND)]
        for d_ in range(ND):
            nc.sync.dma_start(out=xs[d_][:], in_=xT[d_ * P:(d_ + 1) * P, :])
        ck = sb.tile([P, SK], bf16, tag="ck", name="ck")
        sk_ = sb.tile([P, SK], bf16, tag="sk", name="sk")
        nc.sync.dma_start(out=ck[:], in_=cosk[:, :])
        nc.sync.dma_start(out=sk_[:], in_=sink[:, :])
        if with_bias:
            xone = sb.tile([1, SK], bf16, tag="xone", name="xone")
            nc.sync.dma_start(out=xone[:], in_=xT[D:D + 1, :])
            wvb = wp.tile([1, D], bf16, tag="wvb", name="wvb", bufs=1)
            nc.sync.dma_start(out=wvb[:], in_=wvT[D:D + 1, :])

        # ---- V projection (natural layout, x^T stationary) -------------
        for t_ in range(NT):
            for oh in range(2):
                ps = psp.tile([P, QF], f32, tag="pp", name="pp")
                for d_ in range(ND):
                    nc.tensor.matmul(
                        ps[:], xs[d_][:, t_ * P:(t_ + 1) * P],
                        wv[:, d_, oh * QF:(oh + 1) * QF],
                        start=(d_ == 0), stop=(not with_bias and d_ == ND - 1))
                if with_bias:
                    nc.tensor.matmul(
                        ps[:], xone[:, t_ * P:(t_ + 1) * P],
                        wvb[:, oh * QF:(oh + 1) * QF],
                        start=False, stop=True)
                nc.scalar.activation(
                    vst[t_][:, oh * 8:(oh + 1) * 8, 0:HD],
                    ps[:].rearrange("p (h d) -> p h d", d=HD), AF.Copy)
            nc.vector.memset(vst[t_][:, :, HD:HD + 1], 1.0)

        def qk_proj(wdram, wtag, dst, ntok):
            """dst[o128, t] = rope(W[pi-slice] @ x^T + b); o-slice weights
            are streamed per head-pair; Q's rope table is the leading
            [*, 0:SQ] slice of K's (own tokens first in the x^T perm)."""
            ws = wp.tile([P, ND, P], bf16, tag=wtag, name=wtag)
            nc.sync.dma_start(
                out=ws[:],
                in_=wdram[0:D, :].rearrange("(n p) o -> p n o", p=P))
            if with_bias:
                wb = wp.tile([1, P], bf16, tag=wtag + "b", name=wtag + "b")
                nc.sync.dma_start(out=wb[:], in_=wdram[D:D + 1, :])
            t2 = tp.tile([P, ntok], bf16, tag="t2", name="t2")
            t2s = tp.tile([P, ntok], bf16, tag="t2s", name="t2s")
            for c in range(ntok // QF):
                ps = psp.tile([P, QF], f32, tag="pp", name="pp")
                for d_ in range(ND):
                    nc.tensor.matmul(
                        ps[:], ws[:, d_, :], xs[d_][:, c * QF:(c + 1) * QF],
                        start=(d_ == 0), stop=(not with_bias and d_ == ND - 1))
                if with_bias:
                    nc.tensor.matmul(
                        ps[:], wb[:], xone[:, c * QF:(c + 1) * QF],
                        start=False, stop=True)
                cslice = slice(c * QF, (c + 1) * QF)
                nc.vector.tensor_mul(dst[:, cslice], ps[:], ck[:, cslice])
                nc.vector.tensor_mul(t2[:, cslice], ps[:], sk_[:, cslice])
            # batched band swap d<->d+32 (scalar engine's DMA queue)
            for b0 in (0, 64):
                nc.scalar.dma_start(
                    out=t2s[b0:b0 + 32, :], in_=t2[b0 + 32:b0 + 64, :])
                nc.scalar.dma_start(
                    out=t2s[b0 + 32:b0 + 64, :], in_=t2[b0:b0 + 32, :])
            nc.vector.tensor_add(dst[:], dst[:], t2s[:])

        # ---- fused per-head-pair projection + attention ----------------
        for pi in range(ND):
            osl = slice(pi * P, (pi + 1) * P)
            qtile = qk.tile([P, SQ], bf16, tag="qt", name="qt")
            qk_proj(wqT[:, osl], "wq", qtile, SQ)
            ktile = qk.tile([P, SK], bf16, tag="kt", name="kt")
            qk_proj(wkT[:, osl], "wk", ktile, SK)

            for qh in range(NQF):
                qs = slice(qh * QF, (qh + 1) * QF)
                ope = psp.tile([HD + 1, QF], f32, tag="o", name="o")
                opo = psp.tile([HD + 1, QF], f32, tag="o", name="o")
                prev = None
                for kcp in range(NT // 2):
                    # even/odd heads on disjoint PE row groups: the T0/T8
                    # pairs execute concurrently in 64-row tiling mode
                    spe = psp.tile([P, 2 * QF], f32, tag="s", name="s")
                    spo = psp.tile([P, 2 * QF], f32, tag="s", name="s")
                    for j in range(2):
                        ks_ = slice((2 * kcp + j) * P, (2 * kcp + j + 1) * P)
                        js = slice(j * QF, (j + 1) * QF)
                        nc.tensor.matmul(
                            spe[:, js], ktile[0:64, ks_], qtile[0:64, qs],
                            start=True, stop=True)
                        nc.tensor.matmul(
                            spo[:, js], ktile[64:128, ks_], qtile[64:128, qs],
                            start=True, stop=True)
                    ee = etp.tile([P, 2 * QF], bf16, tag="e", name="e")
                    eo = etp.tile([P, 2 * QF], bf16, tag="e", name="e")
                    nc.scalar.activation(ee[:], spe[:], AF.Exp, scale=0.125)
                    nc.scalar.activation(eo[:], spo[:], AF.Exp, scale=0.125)
                    # attn@V for the previous k-chunk pair overlaps this
                    # pair's exp latency on the PE
                    if prev is not None:
                        pee, peo, pk = prev
                        for j in range(2):
                            kc = 2 * pk + j
                            js = slice(j * QF, (j + 1) * QF)
                            nc.tensor.matmul(
                                ope[:], vst[kc][:, 2 * pi, :], pee[:, js],
                                start=(kc == 0), stop=False)
                            nc.tensor.matmul(
                                opo[:], vst[kc][:, 2 * pi + 1, :], peo[:, js],
                                start=(kc == 0), stop=False)
                    prev = (ee, eo, kcp)
                pee, peo, pk = prev
                for j in range(2):
                    kc = 2 * pk + j
                    js = slice(j * QF, (j + 1) * QF)
                    nc.tensor.matmul(
                        ope[:], vst[kc][:, 2 * pi, :], pee[:, js],
                        start=False, stop=(kc == NT - 1))
                    nc.tensor.matmul(
                        opo[:], vst[kc][:, 2 * pi + 1, :], peo[:, js],
                        start=False, stop=(kc == NT - 1))

                for h, op in ((2 * pi, ope), (2 * pi + 1, opo)):
                    # evict PSUM immediately (DVE), then normalize in SBUF:
                    # exact reciprocal base-aligned at p64, gpsimd DMA-hop
                    # to p0, partition-broadcast, multiply
                    osb = npool.tile([HD + 1, QF], f32, tag="osb", name="osb")
                    nc.vector.tensor_copy(osb[:], op[:])
                    sm = npool.tile([HD + 1, QF], f32, tag="sm", name="sm")
                    nc.vector.reciprocal(sm[HD:HD + 1, :], osb[HD:HD + 1, :])
                    rc = npool.tile([1, QF], f32, tag="rc", name="rc")
                    nc.gpsimd.dma_start(out=rc[:], in_=sm[HD:HD + 1, :])
                    bc = npool.tile([HD, QF], f32, tag="bc", name="bc")
                    nc.gpsimd.partition_broadcast(bc[:], rc[:])
                    if h % 2 == 0:
                        nc.vector.tensor_mul(
                            at[pi][0:64, qs], osb[0:HD, :], bc[:])
                    else:
                        # odd heads land at partition base 64; a pure-SBUF
                        # base-shifted DVE write corrupts, so write at base
                        # 0 and DMA-hop into place
                        atm = npool.tile([HD, QF], bf16, tag="atm", name="atm")
                        nc.vector.tensor_mul(atm[:], osb[0:HD, :], bc[:])
                        nc.gpsimd.dma_start(out=at[pi][64:128, qs], in_=atm[:])

        # ---- o-projection ---------------------------------------------
        wo = wp.tile([P, ND, D], bf16, tag="wbig", name="wo", bufs=1)
        nc.sync.dma_start(
            out=wo[:], in_=woT[:, :].rearrange("(n p) o -> p n o", p=P))
        for qc in range(ND):
            for oh in range(2):
                ps = psp.tile([P, QF], f32, tag="pp", name="pp")
                for f in range(ND):
                    nc.tensor.matmul(
                        ps[:], at[f][:, qc * P:(qc + 1) * P],
                        wo[:, f, oh * QF:(oh + 1) * QF],
                        start=(f == 0), stop=(f == ND - 1))
                ob = outp.tile([P, QF], f32, tag="ob", name="ob")
                nc.scalar.activation(ob[:], ps[:], AF.Copy)
                nc.sync.dma_start(
                    out=out[qc * P:(qc + 1) * P, oh * QF:(oh + 1) * QF],
                    in_=ob[:])
    nc.compile()
    return nc


def _rope_tables(pos):
    """pos [n] -> (cos [128, n] bf16, sign-folded sin [128, n] bf16)."""
    inv = ROPE_BASE ** (-np.arange(0, HD, 2, dtype=np.float64) / HD)
    fr = np.outer(pos.astype(np.float64), inv)          # [n, 32]
    c, s = np.cos(fr), np.sin(fr)
    cos64 = np.concatenate([c, c], axis=1).T            # [64, n]
    sinA = np.concatenate([s, -s], axis=1).T            # [64, n]
    return (np.tile(cos64, (2, 1)).astype(BF16),
            np.tile(sinA, (2, 1)).astype(BF16))


def _aug_w(w, b):
    """[D, D] weight + [D] bias -> bf16 [D+1, D] (W.T with bias row)."""
    wa = np.empty((D + 1, D), dtype=np.float32)
    wa[:D] = np.asarray(w, dtype=np.float32).T
    wa[D] = np.asarray(b, dtype=np.float32)
    return np.ascontiguousarray(wa).astype(BF16)


def kernel(hidden_states, position_ids, Wq, bq, Wk, bk, Wv, bv, Wo):
    from concourse import bass_utils

    with_bias = bool(
        np.any(np.asarray(bq)) or np.any(np.asarray(bk)) or np.any(np.asarray(bv)))
    key = ("nc", with_bias)
    if key not in _cache:
        _cache[key] = _build_nc(with_bias)
    nc = _cache[key]

    hs = np.asarray(hidden_states, dtype=np.float32)
    pos = np.asarray(position_ids)
    wq = _aug_w(Wq, bq)
    wk = _aug_w(Wk, bk)
    wv = _aug_w(Wv, bv)
    wo = np.ascontiguousarray(np.asarray(Wo, dtype=np.float32).T).astype(BF16)

    in_maps = []
    for core in range(NCORES):
        b, hf = core // 2, core % 2
        perm = np.concatenate([
            np.arange(hf * SQ, (hf + 1) * SQ),
            np.arange((1 - hf) * SQ, (2 - hf) * SQ)])
        xp = hs[b][perm]                                 # [S, D], own half first
        xT = np.empty((D + 1, SK), dtype=np.float32)
        xT[:D] = xp.T
        xT[D] = 1.0
        ck, sk = _rope_tables(np.asarray(pos[b][perm]))
        in_maps.append({
            "xT": xT.astype(BF16), "wqT": wq, "wkT": wk, "wvT": wv, "woT": wo,
            "cosk": ck, "sink": sk,
        })

    res = bass_utils.run_bass_kernel_spmd(
        nc, in_maps, core_ids=list(range(NCORES)), trace=TRACE, **TRACE_KW)
    LAST["exec_time_ns"] = res.exec_time_ns
    LAST["mean_exec_time_ns"] = res.mean_exec_time_ns
    LAST["trace"] = res.instructions_and_trace
    LAST["profile_json"] = res.profile_json

    outp_full = np.empty((B, S, D), dtype=np.float32)
    for core in range(NCORES):
        b, hf = core // 2, core % 2
        outp_full[b, hf * SQ:(hf + 1) * SQ] = res.results[core]["out"]
    return outp_full


# revision 14
# speedup vs baseline: 1.1550x; 1.0970x over previous
"""Distributed Trainium2 Bass kernel for multi-head attention w/ RoPE.

Reference op (B=4, S=2048, D=1024, H=16, HD=64, fp32):
    q/k/v = hidden @ W{q,k,v}.T + b   (per-head reshape)
    q, k  = rope(q), rope(k)
    out   = softmax(q k^T / sqrt(HD)) v  @ Wo.T

Sharding: 8 cores = 4 batches x 2 query-halves. Each core computes the
K/V projections for its whole batch (duplicated across the half-pair --
this avoids every collective), Q projection + attention for its own 1024
queries, and the o-projection for its own output rows. Host-side unshard
is a pure concat. Per-core x^T is column-permuted so the core's own
queries always sit at columns 0:1024 (keeps the SPMD graph uniform);
K/V/rope tables follow the same permutation, which softmax+V is
invariant to.

Single fused pipeline, fully transposed layout (features on partitions):
V projects first (natural layout, ones column appended so the softmax
denominator falls out of the attn@V matmul); then per head-pair: Q^T/K^T
projection chunks -> RoPE (DVE muls + a batched DMA partition band-swap
+ one 2x bf16 add) -> scores S^T with even/odd heads issued on disjoint
PE row-groups (64-row tiling, concurrent) -> wide [128,1024] exp on ACT
with the 1/sqrt(HD) scale folded in -> attn@V accumulation interleaved
at k-chunk granularity so PE fills ACT's exp latency. Normalization is
evicted-early (DVE copy frees PSUM), exact reciprocal runs base-aligned,
gpsimd hops/broadcasts it, and odd heads write via a DMA partition hop.
The o-projection consumes the transposed attention output directly.
Nonzero biases ride an augmented K=1 contraction row (skipped when the
caller's biases are all zero). All matmuls bf16, fp32 accumulation.
"""

import sys

import numpy as np

try:  # concourse ships in the container; fall back to the staged repo
    import concourse.bass  # noqa: F401
except Exception:  # pragma: no cover
    sys.path.insert(0, "/opt/trn_rl_repo")

import ml_dtypes

B, S, D, H = 4, 2048, 1024, 16
HD = D // H                      # 64
P = 128
NCORES = 8
SQ = S // 2                      # 1024 queries per core
SK = S                           # 2048 keys per core
ND = D // P                      # 8 feature chunks
NT = SK // P                     # 16 key/token chunks
QF = 512                         # matmul moving width
NQF = SQ // QF                   # 2
ROPE_BASE = 10000.0
BF16 = ml_dtypes.bfloat16

TRACE = False                    # test harness flips this
TRACE_KW = {}
LAST = {}                        # exec_time_ns / trace path for test harness

_cache = {}


def _build_nc(with_bias):
    import concourse.bass as bass
    import concourse.mybir as mybir
    import concourse.tile as tile
    from concourse import bacc
    from contextlib import ExitStack

    f32 = mybir.dt.float32
    bf16 = mybir.dt.bfloat16
    AF = mybir.ActivationFunctionType
    PSUM = bass.MemorySpace.PSUM

    nc = bacc.Bacc(None)
    xT = nc.declare_dram_parameter("xT", [D + 1, SK], bf16, False)
    wqT = nc.declare_dram_parameter("wqT", [D + 1, D], bf16, False)
    wkT = nc.declare_dram_parameter("wkT", [D + 1, D], bf16, False)
    wvT = nc.declare_dram_parameter("wvT", [D + 1, D], bf16, False)
    woT = nc.declare_dram_parameter("woT", [D, D], bf16, False)
    cosk = nc.declare_dram_parameter("cosk", [P, SK], bf16, False)
    sink = nc.declare_dram_parameter("sink", [P, SK], bf16, False)
    out = nc.declare_dram_parameter("out", [SQ, D], f32, True)

    with tile.TileContext(nc) as tc, ExitStack() as st:
        sb = st.enter_context(tc.tile_pool(name="sb", bufs=1))
        qk = st.enter_context(tc.tile_pool(name="qk", bufs=3))
        wp = st.enter_context(tc.tile_pool(name="wp", bufs=2))
        tp = st.enter_context(tc.tile_pool(name="tp", bufs=2))
        etp = st.enter_context(tc.tile_pool(name="et", bufs=6))
        npool = st.enter_context(tc.tile_pool(name="nrm", bufs=3))
        outp = st.enter_context(tc.tile_pool(name="ou", bufs=2))
        psp = st.enter_context(tc.tile_pool(name="ps", bufs=2, space=PSUM))

        vst = [sb.tile([P, H, HD + 1], bf16, tag=f"v{t}", name=f"v{t}")
               for t in range(NT)]
        at = [sb.tile([P, SQ], bf16, tag=f"at{i}", name=f"at{i}")
              for i in range(ND)]

        # ---- loads -----------------------------------------------------
        wv = wp.tile([P, ND, D], bf16, tag="wbig", name="wv", bufs=1)
        for d_ in range(ND):
            nc.sync.dma_start(out=wv[:, d_, :], in_=wvT[d_ * P:(d_ + 1) * P, :])
        xs = [sb.tile([P, SK], bf16, tag=f"x{d}", name=f"x{d}")
              for d in range(ND)]
        for d_ in range(ND):
            nc.sync.dma_start(out=xs[d_][:], in_=xT[d_ * P:(d_ + 1) * P, :])
        ck = sb.tile([P, SK], bf16, tag="ck", name="ck")
        sk_ = sb.tile([P, SK], bf16, tag="sk", name="sk")
        nc.sync.dma_start(out=ck[:], in_=cosk[:, :])
        nc.sync.dma_start(out=sk_[:], in_=sink[:, :])
        if with_bias:
            xone = sb.tile([1, SK], bf16, tag="xone", name="xone")
            nc.sync.dma_start(out=xone[:], in_=xT[D:D + 1, :])
            wvb = wp.tile([1, D], bf16, tag="wvb", name="wvb", bufs=1)
            nc.sync.dma_start(out=wvb[:], in_=wvT[D:D + 1, :])

        # ---- V projection (natural layout, x^T stationary) -------------
        for t_ in range(NT):
            for oh in range(2):
                ps = psp.tile([P, QF], f32, tag="pp", name="pp")
                for d_ in range(ND):
                    nc.tensor.matmul(
                        ps[:], xs[d_][:, t_ * P:(t_ + 1) * P],
                        wv[:, d_, oh * QF:(oh + 1) * QF],
                        start=(d_ == 0), stop=(not with_bias and d_ == ND - 1))
                if with_bias:
                    nc.tensor.matmul(
                        ps[:], xone[:, t_ * P:(t_ + 1) * P],
                        wvb[:, oh * QF:(oh + 1) * QF],
                        start=False, stop=True)
                nc.scalar.activation(
                    vst[t_][:, oh * 8:(oh + 1) * 8, 0:HD],
                    ps[:].rearrange("p (h d) -> p h d", d=HD), AF.Copy)
            nc.vector.memset(vst[t_][:, :, HD:HD + 1], 1.0)

        def load_wslice(wdram, wtag):
            ws = wp.tile([P, ND, P], bf16, tag=wtag, name=wtag)
            nc.sync.dma_start(
                out=ws[:],
                in_=wdram[0:D, :].rearrange("(n p) o -> p n o", p=P))
            wb = None
            if with_bias:
                wb = wp.tile([1, P], bf16, tag=wtag + "b", name=wtag + "b")
                nc.sync.dma_start(out=wb[:], in_=wdram[D:D + 1, :])
            return ws, wb

        def qk_proj(wsb, dst, ntok):
            """dst[o128, t] = rope(W[pi-slice] @ x^T + b); rope swap+add
            runs per 1024-wide half so scores can start on half 0. Q's
            rope table is the leading [*, 0:SQ] slice of K's (own tokens
            first in the x^T perm)."""
            ws, wb = wsb
            t2 = tp.tile([P, ntok], bf16, tag="t2", name="t2")
            t2s = tp.tile([P, ntok], bf16, tag="t2s", name="t2s")
            for c in range(ntok // QF):
                ps = psp.tile([P, QF], f32, tag="pp", name="pp")
                for d_ in range(ND):
                    nc.tensor.matmul(
                        ps[:], ws[:, d_, :], xs[d_][:, c * QF:(c + 1) * QF],
                        start=(d_ == 0), stop=(not with_bias and d_ == ND - 1))
                if with_bias:
                    nc.tensor.matmul(
                        ps[:], wb[:], xone[:, c * QF:(c + 1) * QF],
                        start=False, stop=True)
                cslice = slice(c * QF, (c + 1) * QF)
                nc.vector.tensor_mul(dst[:, cslice], ps[:], ck[:, cslice])
                nc.vector.tensor_mul(t2[:, cslice], ps[:], sk_[:, cslice])
                if c % 2 == 1:
                    # band swap d<->d+32 (scalar engine's DMA queue) + add,
                    # batched over the finished 1024-wide half
                    hs_ = slice((c - 1) * QF, (c + 1) * QF)
                    for b0 in (0, 64):
                        nc.scalar.dma_start(
                            out=t2s[b0:b0 + 32, hs_], in_=t2[b0 + 32:b0 + 64, hs_])
                        nc.scalar.dma_start(
                            out=t2s[b0 + 32:b0 + 64, hs_], in_=t2[b0:b0 + 32, hs_])
                    nc.vector.tensor_add(
                        dst[:, hs_], dst[:, hs_], t2s[:, hs_])

        # ---- fused per-head-pair projection + attention ----------------
        wnext = (load_wslice(wqT[:, 0:P], "wq"), load_wslice(wkT[:, 0:P], "wk"))
        for pi in range(ND):
            wcur = wnext
            if pi + 1 < ND:
                osl = slice((pi + 1) * P, (pi + 2) * P)
                wnext = (load_wslice(wqT[:, osl], "wq"),
                         load_wslice(wkT[:, osl], "wk"))
            qtile = qk.tile([P, SQ], bf16, tag="qt", name="qt")
            qk_proj(wcur[0], qtile, SQ)
            ktile = qk.tile([P, SK], bf16, tag="kt", name="kt")
            qk_proj(wcur[1], ktile, SK)

            for qh in range(NQF):
                qs = slice(qh * QF, (qh + 1) * QF)
                ope = psp.tile([HD + 1, QF], f32, tag="o", name="o")
                opo = psp.tile([HD + 1, QF], f32, tag="o", name="o")
                prev = None
                for kcp in range(NT // 2):
                    # even/odd heads on disjoint PE row groups: the T0/T8
                    # pairs execute concurrently in 64-row tiling mode
                    spe = psp.tile([P, 2 * QF], f32, tag="s", name="s")
                    spo = psp.tile([P, 2 * QF], f32, tag="s", name="s")
                    for j in range(2):
                        ks_ = slice((2 * kcp + j) * P, (2 * kcp + j + 1) * P)
                        js = slice(j * QF, (j + 1) * QF)
                        nc.tensor.matmul(
                            spe[:, js], ktile[0:64, ks_], qtile[0:64, qs],
                            start=True, stop=True)
                        nc.tensor.matmul(
                            spo[:, js], ktile[64:128, ks_], qtile[64:128, qs],
                            start=True, stop=True)
                    ee = etp.tile([P, 2 * QF], bf16, tag="e", name="e")
                    eo = etp.tile([P, 2 * QF], bf16, tag="e", name="e")
                    nc.scalar.activation(ee[:], spe[:], AF.Exp, scale=0.125)
                    nc.scalar.activation(eo[:], spo[:], AF.Exp, scale=0.125)
                    # attn@V for the previous k-chunk pair overlaps this
                    # pair's exp latency on the PE
                    if prev is not None:
                        pee, peo, pk = prev
                        for j in range(2):
                            kc = 2 * pk + j
                            js = slice(j * QF, (j + 1) * QF)
                            nc.tensor.matmul(
                                ope[:], vst[kc][:, 2 * pi, :], pee[:, js],
                                start=(kc == 0), stop=False)
                            nc.tensor.matmul(
                                opo[:], vst[kc][:, 2 * pi + 1, :], peo[:, js],
                                start=(kc == 0), stop=False)
                    prev = (ee, eo, kcp)
                pee, peo, pk = prev
                for j in range(2):
                    kc = 2 * pk + j
                    js = slice(j * QF, (j + 1) * QF)
                    nc.tensor.matmul(
                        ope[:], vst[kc][:, 2 * pi, :], pee[:, js],
                        start=False, stop=(kc == NT - 1))
                    nc.tensor.matmul(
                        opo[:], vst[kc][:, 2 * pi + 1, :], peo[:, js],
                        start=False, stop=(kc == NT - 1))

                for h, op in ((2 * pi, ope), (2 * pi + 1, opo)):
                    # evict PSUM immediately (DVE), then normalize in SBUF:
                    # exact reciprocal base-aligned at p64, gpsimd DMA-hop
                    # to p0, partition-broadcast, multiply
                    osb = npool.tile([HD + 1, QF], f32, tag="osb", name="osb")
                    nc.vector.tensor_copy(osb[:], op[:])
                    sm = npool.tile([HD + 1, QF], f32, tag="sm", name="sm")
                    nc.vector.reciprocal(sm[HD:HD + 1, :], osb[HD:HD + 1, :])
                    rc = npool.tile([1, QF], f32, tag="rc", name="rc")
                    nc.gpsimd.dma_start(out=rc[:], in_=sm[HD:HD + 1, :])
                    bc = npool.tile([HD, QF], f32, tag="bc", name="bc")
                    nc.gpsimd.partition_broadcast(bc[:], rc[:])
                    if h % 2 == 0:
                        nc.vector.tensor_mul(
                            at[pi][0:64, qs], osb[0:HD, :], bc[:])
                    else:
                        # odd heads land at partition base 64; a pure-SBUF
                        # base-shifted DVE write corrupts, so write at base
                        # 0 and DMA-hop into place
                        atm = npool.tile([HD, QF], bf16, tag="atm", name="atm")
                        nc.vector.tensor_mul(atm[:], osb[0:HD, :], bc[:])
                        nc.gpsimd.dma_start(out=at[pi][64:128, qs], in_=atm[:])

        # ---- o-projection ---------------------------------------------
        wo = wp.tile([P, ND, D], bf16, tag="wbig", name="wo", bufs=1)
        for d_ in range(ND):
            nc.sync.dma_start(out=wo[:, d_, :], in_=woT[d_ * P:(d_ + 1) * P, :])
        for qc in range(ND):
            for oh in range(2):
                ps = psp.tile([P, QF], f32, tag="pp", name="pp")
                for f in range(ND):
                    nc.tensor.matmul(
                        ps[:], at[f][:, qc * P:(qc + 1) * P],
                        wo[:, f, oh * QF:(oh + 1) * QF],
                        start=(f == 0), stop=(f == ND - 1))
                ob = outp.tile([P, QF], f32, tag="ob", name="ob")
                nc.scalar.activation(ob[:], ps[:], AF.Copy)
                nc.sync.dma_start(
                    out=out[qc * P:(qc + 1) * P, oh * QF:(oh + 1) * QF],
                    in_=ob[:])
    nc.compile()
    return nc


def _rope_tables(pos):
    """pos [n] -> (cos [128, n] bf16, sign-folded sin [128, n] bf16)."""
    inv = ROPE_BASE ** (-np.arange(0, HD, 2, dtype=np.float64) / HD)
    fr = np.outer(pos.astype(np.float64), inv)          # [n, 32]
    c, s = np.cos(fr), np.sin(fr)
    cos64 = np.concatenate([c, c], axis=1).T            # [64, n]
    sinA = np.concatenate([s, -s], axis=1).T            # [64, n]
    return (np.tile(cos64, (2, 1)).astype(BF16),
            np.tile(sinA, (2, 1)).astype(BF16))


def _aug_w(w, b):
    """[D, D] weight + [D] bias -> bf16 [D+1, D] (W.T with bias row)."""
    wa = np.empty((D + 1, D), dtype=np.float32)
    wa[:D] = np.asarray(w, dtype=np.float32).T
    wa[D] = np.asarray(b, dtype=np.float32)
    return np.ascontiguousarray(wa).astype(BF16)


def kernel(hidden_states, position_ids, Wq, bq, Wk, bk, Wv, bv, Wo):
    from concourse import bass_utils

    with_bias = bool(
        np.any(np.asarray(bq)) or np.any(np.asarray(bk)) or np.any(np.asarray(bv)))
    key = ("nc", with_bias)
    if key not in _cache:
        _cache[key] = _build_nc(with_bias)
    nc = _cache[key]

    hs = np.asarray(hidden_states, dtype=np.float32)
    pos = np.asarray(position_ids)
    wq = _aug_w(Wq, bq)
    wk = _aug_w(Wk, bk)
    wv = _aug_w(Wv, bv)
    wo = np.ascontiguousarray(np.asarray(Wo, dtype=np.float32).T).astype(BF16)

    in_maps = []
    for core in range(NCORES):
        b, hf = core // 2, core % 2
        perm = np.concatenate([
            np.arange(hf * SQ, (hf + 1) * SQ),
            np.arange((1 - hf) * SQ, (2 - hf) * SQ)])
        xp = hs[b][perm]                                 # [S, D], own half first
        xT = np.empty((D + 1, SK), dtype=np.float32)
        xT[:D] = xp.T
        xT[D] = 1.0
        ck, sk = _rope_tables(np.asarray(pos[b][perm]))
        in_maps.append({
            "xT": xT.astype(BF16), "wqT": wq, "wkT": wk, "wvT": wv, "woT": wo,
            "cosk": ck, "sink": sk,
        })

    res = bass_utils.run_bass_kernel_spmd(
        nc, in_maps, core_ids=list(range(NCORES)), trace=TRACE, **TRACE_KW)
    LAST["exec_time_ns"] = res.exec_time_ns
    LAST["mean_exec_time_ns"] = res.mean_exec_time_ns
    LAST["trace"] = res.instructions_and_trace
    LAST["profile_json"] = res.profile_json

    outp_full = np.empty((B, S, D), dtype=np.float32)
    for core in range(NCORES):
        b, hf = core // 2, core % 2
        outp_full[b, hf * SQ:(hf + 1) * SQ] = res.results[core]["out"]
    return outp_full


# revision 15
# speedup vs baseline: 1.1584x; 1.0029x over previous
"""Distributed Trainium2 Bass kernel for multi-head attention w/ RoPE.

Reference op (B=4, S=2048, D=1024, H=16, HD=64, fp32):
    q/k/v = hidden @ W{q,k,v}.T + b   (per-head reshape)
    q, k  = rope(q), rope(k)
    out   = softmax(q k^T / sqrt(HD)) v  @ Wo.T

Sharding: 8 cores = 4 batches x 2 query-halves. Each core computes the
K/V projections for its whole batch (duplicated across the half-pair --
this avoids every collective), Q projection + attention for its own 1024
queries, and the o-projection for its own output rows. Host-side unshard
is a pure concat. Per-core x^T is column-permuted so the core's own
queries always sit at columns 0:1024 (keeps the SPMD graph uniform);
K/V/rope tables follow the same permutation, which softmax+V is
invariant to.

Single fused pipeline, fully transposed layout (features on partitions):
V projects first (natural layout, ones column appended so the softmax
denominator falls out of the attn@V matmul); then per head-pair: Q^T/K^T
projection chunks -> RoPE (DVE muls + a batched DMA partition band-swap
+ one 2x bf16 add) -> scores S^T with even/odd heads issued on disjoint
PE row-groups (64-row tiling, concurrent) -> wide [128,1024] exp on ACT
with the 1/sqrt(HD) scale folded in -> attn@V accumulation interleaved
at k-chunk granularity so PE fills ACT's exp latency. Normalization is
evicted-early (DVE copy frees PSUM), exact reciprocal runs base-aligned,
gpsimd hops/broadcasts it, and odd heads write via a DMA partition hop.
The o-projection consumes the transposed attention output directly.
Nonzero biases ride an augmented K=1 contraction row (skipped when the
caller's biases are all zero). All matmuls bf16, fp32 accumulation.
"""

import sys

import numpy as np

try:  # concourse ships in the container; fall back to the staged repo
    import concourse.bass  # noqa: F401
except Exception:  # pragma: no cover
    sys.path.insert(0, "/opt/trn_rl_repo")

import ml_dtypes

B, S, D, H = 4, 2048, 1024, 16
HD = D // H                      # 64
P = 128
NCORES = 8
SQ = S // 2                      # 1024 queries per core
SK = S                           # 2048 keys per core
ND = D // P                      # 8 feature chunks
NT = SK // P                     # 16 key/token chunks
QF = 512                         # matmul moving width
NQF = SQ // QF                   # 2
ROPE_BASE = 10000.0
BF16 = ml_dtypes.bfloat16

TRACE = False                    # test harness flips this
TRACE_KW = {}
LAST = {}                        # exec_time_ns / trace path for test harness

_cache = {}


def _build_nc(with_bias):
    import concourse.bass as bass
    import concourse.mybir as mybir
    import concourse.tile as tile
    from concourse import bacc
    from contextlib import ExitStack

    f32 = mybir.dt.float32
    bf16 = mybir.dt.bfloat16
    AF = mybir.ActivationFunctionType
    PSUM = bass.MemorySpace.PSUM

    nc = bacc.Bacc(None)
    xT = nc.declare_dram_parameter("xT", [D + 1, SK], bf16, False)
    wqT = nc.declare_dram_parameter("wqT", [D + 1, D], bf16, False)
    wkT = nc.declare_dram_parameter("wkT", [D + 1, D], bf16, False)
    wvT = nc.declare_dram_parameter("wvT", [D + 1, D], bf16, False)
    woT = nc.declare_dram_parameter("woT", [D, D], bf16, False)
    cosk = nc.declare_dram_parameter("cosk", [P, SK], bf16, False)
    sink = nc.declare_dram_parameter("sink", [P, SK], bf16, False)
    out = nc.declare_dram_parameter("out", [SQ, D], f32, True)

    with tile.TileContext(nc) as tc, ExitStack() as st:
        sb = st.enter_context(tc.tile_pool(name="sb", bufs=1))
        qk = st.enter_context(tc.tile_pool(name="qk", bufs=3))
        wp = st.enter_context(tc.tile_pool(name="wp", bufs=2))
        tp = st.enter_context(tc.tile_pool(name="tp", bufs=2))
        etp = st.enter_context(tc.tile_pool(name="et", bufs=6))
        npool = st.enter_context(tc.tile_pool(name="nrm", bufs=3))
        outp = st.enter_context(tc.tile_pool(name="ou", bufs=2))
        psp = st.enter_context(tc.tile_pool(name="ps", bufs=2, space=PSUM))

        vst = [sb.tile([P, H, HD + 1], bf16, tag=f"v{t}", name=f"v{t}")
               for t in range(NT)]
        at = [sb.tile([P, SQ], bf16, tag=f"at{i}", name=f"at{i}")
              for i in range(ND)]

        # ---- loads -----------------------------------------------------
        wv = wp.tile([P, ND, D], bf16, tag="wbig", name="wv", bufs=1)
        for d_ in range(ND):
            nc.sync.dma_start(out=wv[:, d_, :], in_=wvT[d_ * P:(d_ + 1) * P, :])
        xs = [sb.tile([P, SK], bf16, tag=f"x{d}", name=f"x{d}")
              for d in range(ND)]
        for d_ in range(ND):
            nc.sync.dma_start(out=xs[d_][:], in_=xT[d_ * P:(d_ + 1) * P, :])
        ck = sb.tile([P, SK], bf16, tag="ck", name="ck")
        sk_ = sb.tile([P, SK], bf16, tag="sk", name="sk")
        nc.sync.dma_start(out=ck[:], in_=cosk[:, :])
        nc.sync.dma_start(out=sk_[:], in_=sink[:, :])
        if with_bias:
            xone = sb.tile([1, SK], bf16, tag="xone", name="xone")
            nc.sync.dma_start(out=xone[:], in_=xT[D:D + 1, :])
            wvb = wp.tile([1, D], bf16, tag="wvb", name="wvb", bufs=1)
            nc.sync.dma_start(out=wvb[:], in_=wvT[D:D + 1, :])

        # ---- V projection (natural layout, x^T stationary) -------------
        for t_ in range(NT):
            for oh in range(2):
                ps = psp.tile([P, QF], f32, tag="pp", name="pp")
                for d_ in range(ND):
                    nc.tensor.matmul(
                        ps[:], xs[d_][:, t_ * P:(t_ + 1) * P],
                        wv[:, d_, oh * QF:(oh + 1) * QF],
                        start=(d_ == 0), stop=(not with_bias and d_ == ND - 1))
                if with_bias:
                    nc.tensor.matmul(
                        ps[:], xone[:, t_ * P:(t_ + 1) * P],
                        wvb[:, oh * QF:(oh + 1) * QF],
                        start=False, stop=True)
                nc.scalar.activation(
                    vst[t_][:, oh * 8:(oh + 1) * 8, 0:HD],
                    ps[:].rearrange("p (h d) -> p h d", d=HD), AF.Copy)
            nc.vector.memset(vst[t_][:, :, HD:HD + 1], 1.0)

        def load_wslice(wdram, wtag):
            ws = wp.tile([P, ND, P], bf16, tag=wtag, name=wtag)
            nc.sync.dma_start(
                out=ws[:],
                in_=wdram[0:D, :].rearrange("(n p) o -> p n o", p=P))
            wb = None
            if with_bias:
                wb = wp.tile([1, P], bf16, tag=wtag + "b", name=wtag + "b")
                nc.sync.dma_start(out=wb[:], in_=wdram[D:D + 1, :])
            return ws, wb

        def qk_proj(wsb, dst, ntok):
            """dst[o128, t] = rope(W[pi-slice] @ x^T + b); rope swap+add
            runs per 1024-wide half so scores can start on half 0. Q's
            rope table is the leading [*, 0:SQ] slice of K's (own tokens
            first in the x^T perm)."""
            ws, wb = wsb
            t2 = tp.tile([P, ntok], bf16, tag="t2", name="t2")
            t2s = tp.tile([P, ntok], bf16, tag="t2s", name="t2s")
            for c in range(ntok // QF):
                ps = psp.tile([P, QF], f32, tag="pp", name="pp")
                for d_ in range(ND):
                    nc.tensor.matmul(
                        ps[:], ws[:, d_, :], xs[d_][:, c * QF:(c + 1) * QF],
                        start=(d_ == 0), stop=(not with_bias and d_ == ND - 1))
                if with_bias:
                    nc.tensor.matmul(
                        ps[:], wb[:], xone[:, c * QF:(c + 1) * QF],
                        start=False, stop=True)
                cslice = slice(c * QF, (c + 1) * QF)
                nc.vector.tensor_mul(dst[:, cslice], ps[:], ck[:, cslice])
                nc.vector.tensor_mul(t2[:, cslice], ps[:], sk_[:, cslice])
                if c % 2 == 1:
                    # band swap d<->d+32 (scalar engine's DMA queue) + add,
                    # batched over the finished 1024-wide half
                    hs_ = slice((c - 1) * QF, (c + 1) * QF)
                    for b0 in (0, 64):
                        nc.scalar.dma_start(
                            out=t2s[b0:b0 + 32, hs_], in_=t2[b0 + 32:b0 + 64, hs_])
                        nc.scalar.dma_start(
                            out=t2s[b0 + 32:b0 + 64, hs_], in_=t2[b0:b0 + 32, hs_])
                    nc.vector.tensor_add(
                        dst[:, hs_], dst[:, hs_], t2s[:, hs_])

        # ---- fused per-head-pair projection + attention ----------------
        pending = []

        def flush_norm():
            # normalize in SBUF: exact reciprocal base-aligned at p64,
            # gpsimd DMA-hop to p0, partition-broadcast, multiply
            for h, ppi, qqs, osb in pending:
                sm = npool.tile([HD + 1, QF], f32, tag="sm", name="sm")
                nc.vector.reciprocal(sm[HD:HD + 1, :], osb[HD:HD + 1, :])
                rc = npool.tile([1, QF], f32, tag="rc", name="rc")
                nc.gpsimd.dma_start(out=rc[:], in_=sm[HD:HD + 1, :])
                bc = npool.tile([HD, QF], f32, tag="bc", name="bc")
                nc.gpsimd.partition_broadcast(bc[:], rc[:])
                if h % 2 == 0:
                    nc.vector.tensor_mul(
                        at[ppi][0:64, qqs], osb[0:HD, :], bc[:])
                else:
                    # odd heads land at partition base 64; a pure-SBUF
                    # base-shifted DVE write corrupts, so write at base
                    # 0 and DMA-hop into place
                    atm = npool.tile([HD, QF], bf16, tag="atm", name="atm")
                    nc.vector.tensor_mul(atm[:], osb[0:HD, :], bc[:])
                    nc.gpsimd.dma_start(out=at[ppi][64:128, qqs], in_=atm[:])
            pending.clear()

        wnext = (load_wslice(wqT[:, 0:P], "wq"), load_wslice(wkT[:, 0:P], "wk"))
        for pi in range(ND):
            wcur = wnext
            if pi + 1 < ND:
                osl = slice((pi + 1) * P, (pi + 2) * P)
                wnext = (load_wslice(wqT[:, osl], "wq"),
                         load_wslice(wkT[:, osl], "wk"))
            qtile = qk.tile([P, SQ], bf16, tag="qt", name="qt")
            qk_proj(wcur[0], qtile, SQ)
            ktile = qk.tile([P, SK], bf16, tag="kt", name="kt")
            qk_proj(wcur[1], ktile, SK)
            flush_norm()

            for qh in range(NQF):
                qs = slice(qh * QF, (qh + 1) * QF)
                ope = psp.tile([HD + 1, QF], f32, tag="o", name="o")
                opo = psp.tile([HD + 1, QF], f32, tag="o", name="o")
                prev = None
                for kcp in range(NT // 2):
                    # even/odd heads on disjoint PE row groups: the T0/T8
                    # pairs execute concurrently in 64-row tiling mode
                    spe = psp.tile([P, 2 * QF], f32, tag="s", name="s")
                    spo = psp.tile([P, 2 * QF], f32, tag="s", name="s")
                    for j in range(2):
                        ks_ = slice((2 * kcp + j) * P, (2 * kcp + j + 1) * P)
                        js = slice(j * QF, (j + 1) * QF)
                        nc.tensor.matmul(
                            spe[:, js], ktile[0:64, ks_], qtile[0:64, qs],
                            start=True, stop=True)
                        nc.tensor.matmul(
                            spo[:, js], ktile[64:128, ks_], qtile[64:128, qs],
                            start=True, stop=True)
                    ee = etp.tile([P, 2 * QF], bf16, tag="e", name="e")
                    eo = etp.tile([P, 2 * QF], bf16, tag="e", name="e")
                    nc.scalar.activation(ee[:], spe[:], AF.Exp, scale=0.125)
                    nc.scalar.activation(eo[:], spo[:], AF.Exp, scale=0.125)
                    # attn@V for the previous k-chunk pair overlaps this
                    # pair's exp latency on the PE
                    if prev is not None:
                        pee, peo, pk = prev
                        for j in range(2):
                            kc = 2 * pk + j
                            js = slice(j * QF, (j + 1) * QF)
                            nc.tensor.matmul(
                                ope[:], vst[kc][:, 2 * pi, :], pee[:, js],
                                start=(kc == 0), stop=False)
                            nc.tensor.matmul(
                                opo[:], vst[kc][:, 2 * pi + 1, :], peo[:, js],
                                start=(kc == 0), stop=False)
                    prev = (ee, eo, kcp)
                pee, peo, pk = prev
                for j in range(2):
                    kc = 2 * pk + j
                    js = slice(j * QF, (j + 1) * QF)
                    nc.tensor.matmul(
                        ope[:], vst[kc][:, 2 * pi, :], pee[:, js],
                        start=False, stop=(kc == NT - 1))
                    nc.tensor.matmul(
                        opo[:], vst[kc][:, 2 * pi + 1, :], peo[:, js],
                        start=False, stop=(kc == NT - 1))

                for h, op in ((2 * pi, ope), (2 * pi + 1, opo)):
                    # evict PSUM immediately (quick DVE copy frees the "o"
                    # slot); the reciprocal chain is emitted one head-pair
                    # later so its 3.3us DVE reciprocals execute while DVE
                    # is otherwise idle and never block PE's PSUM evicts
                    osb = npool.tile([HD + 1, QF], f32, tag="osb", name="osb",
                                     bufs=10)
                    nc.vector.tensor_copy(osb[:], op[:])
                    pending.append((h, pi, qs, osb))

        flush_norm()

        # ---- o-projection ---------------------------------------------
        wo = wp.tile([P, ND, D], bf16, tag="wbig", name="wo", bufs=1)
        for d_ in range(ND):
            nc.sync.dma_start(out=wo[:, d_, :], in_=woT[d_ * P:(d_ + 1) * P, :])
        for qc in range(ND):
            for oh in range(2):
                ps = psp.tile([P, QF], f32, tag="pp", name="pp")
                for f in range(ND):
                    nc.tensor.matmul(
                        ps[:], at[f][:, qc * P:(qc + 1) * P],
                        wo[:, f, oh * QF:(oh + 1) * QF],
                        start=(f == 0), stop=(f == ND - 1))
                ob = outp.tile([P, QF], f32, tag="ob", name="ob")
                nc.scalar.activation(ob[:], ps[:], AF.Copy)
                nc.sync.dma_start(
                    out=out[qc * P:(qc + 1) * P, oh * QF:(oh + 1) * QF],
                    in_=ob[:])
    nc.compile()
    return nc


def _rope_tables(pos):
    """pos [n] -> (cos [128, n] bf16, sign-folded sin [128, n] bf16)."""
    inv = ROPE_BASE ** (-np.arange(0, HD, 2, dtype=np.float64) / HD)
    fr = np.outer(pos.astype(np.float64), inv)          # [n, 32]
    c, s = np.cos(fr), np.sin(fr)
    cos64 = np.concatenate([c, c], axis=1).T            # [64, n]
    sinA = np.concatenate([s, -s], axis=1).T            # [64, n]
    return (np.tile(cos64, (2, 1)).astype(BF16),
            np.tile(sinA, (2, 1)).astype(BF16))


def _aug_w(w, b):
    """[D, D] weight + [D] bias -> bf16 [D+1, D] (W.T with bias row)."""
    wa = np.empty((D + 1, D), dtype=np.float32)
    wa[:D] = np.asarray(w, dtype=np.float32).T
    wa[D] = np.asarray(b, dtype=np.float32)
    return np.ascontiguousarray(wa).astype(BF16)


def kernel(hidden_states, position_ids, Wq, bq, Wk, bk, Wv, bv, Wo):
    from concourse import bass_utils

    with_bias = bool(
        np.any(np.asarray(bq)) or np.any(np.asarray(bk)) or np.any(np.asarray(bv)))
    key = ("nc", with_bias)
    if key not in _cache:
        _cache[key] = _build_nc(with_bias)
    nc = _cache[key]

    hs = np.asarray(hidden_states, dtype=np.float32)
    pos = np.asarray(position_ids)
    wq = _aug_w(Wq, bq)
    wk = _aug_w(Wk, bk)
    wv = _aug_w(Wv, bv)
    wo = np.ascontiguousarray(np.asarray(Wo, dtype=np.float32).T).astype(BF16)

    in_maps = []
    for core in range(NCORES):
        b, hf = core // 2, core % 2
        perm = np.concatenate([
            np.arange(hf * SQ, (hf + 1) * SQ),
            np.arange((1 - hf) * SQ, (2 - hf) * SQ)])
        xp = hs[b][perm]                                 # [S, D], own half first
        xT = np.empty((D + 1, SK), dtype=np.float32)
        xT[:D] = xp.T
        xT[D] = 1.0
        ck, sk = _rope_tables(np.asarray(pos[b][perm]))
        in_maps.append({
            "xT": xT.astype(BF16), "wqT": wq, "wkT": wk, "wvT": wv, "woT": wo,
            "cosk": ck, "sink": sk,
        })

    res = bass_utils.run_bass_kernel_spmd(
        nc, in_maps, core_ids=list(range(NCORES)), trace=TRACE, **TRACE_KW)
    LAST["exec_time_ns"] = res.exec_time_ns
    LAST["mean_exec_time_ns"] = res.mean_exec_time_ns
    LAST["trace"] = res.instructions_and_trace
    LAST["profile_json"] = res.profile_json

    outp_full = np.empty((B, S, D), dtype=np.float32)
    for core in range(NCORES):
        b, hf = core // 2, core % 2
        outp_full[b, hf * SQ:(hf + 1) * SQ] = res.results[core]["out"]
    return outp_full
